# revision 2
# baseline (speedup 1.0000x reference)
"""AnchorGCN layer on 8 TRN2 NeuronCores.

reference:
    support = input @ W.T                         # [N, F]
    anchor_diff = adj / (colsum(adj) + eps)       # [N, A]
    node_diff   = adj / (rowsum(adj) + eps)       # [N, A]
    out = node_diff @ (anchor_diff.T @ support)   # [N, F]

Distributed formulation (rows of input/adj sharded across 8 cores):
    Q    = adj_shard.T @ [input_shard | 1]        # [A, F+1] per-core partial
           (col F of Q is the per-core colsum partial)
    Q    = AllReduce(Q)                           # only collective: 526 KB
    msg  = (Q[:, :F] @ W.T) / (Q[:, F:] + eps)    # [A, F], computed per core
    outp = adj_shard @ [msg | 1]                  # col F = rowsum
    out  = outp[:, :F] / (outp[:, F:] + eps)

Matmuls run in bf16 (f32 PSUM accumulation); normalizations in f32.
adj is transposed on-chip via TensorE (needed as stationary operand for
the final matmul, which contracts over the anchor axis).
"""

import numpy as np

import concourse.bacc as bacc
import concourse.mybir as mybir
import concourse.tile as tile
from concourse.bass_utils import run_bass_kernel_spmd
from concourse.masks import make_identity

F32 = mybir.dt.float32
BF16 = mybir.dt.bfloat16

N, A, F = 50000, 500, 256
EPS = 1e-12
CORES = 8
P = 128
APAD = 512            # anchors padded 500 -> 512 (4 chunks of 128)
FA = F + 1            # input/msg get a ones column appended
T_FULL = 49           # node tiles per core: 8*49*128 = 50176 >= 50000
ACH = APAD // P       # 4 anchor chunks


def build(n_tiles: int = T_FULL, n_cores: int = CORES):
    nt = n_tiles
    rows = nt * P
    nc = bacc.Bacc("TRN2", target_bir_lowering=False, debug=False,
                   num_devices=n_cores)

    inp_d = nc.dram_tensor("input", [rows, FA], F32, kind="ExternalInput")
    adj_d = nc.dram_tensor("adj", [rows, APAD], F32, kind="ExternalInput")
    w_d = nc.dram_tensor("W", [F, F], F32, kind="ExternalInput")
    out_d = nc.dram_tensor("out", [rows, F], F32, kind="ExternalOutput")

    with tile.TileContext(nc) as tc:
        _build_tc(tc, nc, inp_d, adj_d, w_d, out_d, nt, n_cores)
    nc.compile()
    return nc


def _build_tc(tc, nc, inp_d, adj_d, w_d, out_d, nt, n_cores):
    ts = lambda i: slice(i * P, (i + 1) * P)

    with tc.tile_pool(name="const", bufs=1) as const, \
         tc.tile_pool(name="persist", bufs=1) as persist, \
         tc.tile_pool(name="dram", bufs=1, space="DRAM") as dram:

        ident = const.tile([P, P], BF16)
        make_identity(nc, ident[:])

        # ---- W -> W^T (bf16), laid out [fi%128, (fi_half, fo)] ----
        w_sb = const.tile([P, 2 * F], F32)
        nc.sync.dma_start(
            out=w_sb[:].rearrange("p (c f) -> p c f", c=2),
            in_=w_d.ap().rearrange("(c p) f -> p c f", p=P),
        )
        w_bf = const.tile([P, 2 * F], BF16)
        nc.scalar.copy(w_bf[:], w_sb[:])
        wt_bf = const.tile([P, 2 * F], BF16)
        w_bf3 = w_bf[:].rearrange("p (c f) -> p c f", c=2)
        wt_bf3 = wt_bf[:].rearrange("p (c f) -> p c f", c=2)
        with tc.tile_pool(name="wt_ps", bufs=2, space="PSUM") as wtp:
            for foh in range(2):
                for fih in range(2):
                    w_ps = wtp.tile([P, P], BF16, tag="w_ps", bufs=2)
                    nc.tensor.transpose(
                        w_ps[:], w_bf3[:, foh, ts(fih)], ident[:])
                    nc.vector.tensor_copy(wt_bf3[:, fih, ts(foh)], w_ps[:])

        # adj^T, bf16, [a%128, (a_chunk, tile, n)]
        adjT = persist.tile([P, ACH * nt * P], BF16)
        adjT4 = adjT[:].rearrange("p (c t n) -> p c t n", c=ACH, t=nt)

        # ---- loop 1: Q partial accumulation + adj transposes ----
        with tc.tile_pool(name="qps", bufs=1, space="PSUM") as qps, \
             tc.tile_pool(name="l1ps", bufs=3, space="PSUM") as l1ps, \
             tc.tile_pool(name="l1", bufs=1) as l1:
            q_ps = [qps.tile([P, FA], F32, tag=f"q{i}", name=f"q_ps{i}")
                    for i in range(ACH)]
            for t in range(nt):
                in_t = l1.tile([P, FA], F32, tag="in_t", bufs=4)
                nc.sync.dma_start(out=in_t[:], in_=inp_d[ts(t), :])
                adj_t = l1.tile([P, APAD], F32, tag="adj_t", bufs=4)
                nc.sync.dma_start(out=adj_t[:], in_=adj_d[ts(t), :])
                in_bf = l1.tile([P, FA], BF16, tag="in_bf", bufs=4)
                nc.scalar.copy(in_bf[:], in_t[:])
                adj_bf = l1.tile([P, APAD], BF16, tag="adj_bf", bufs=4)
                nc.scalar.copy(adj_bf[:], adj_t[:])
                for ac in range(ACH):
                    nc.tensor.matmul(
                        q_ps[ac][:], adj_bf[:, ts(ac)], in_bf[:],
                        start=(t == 0), stop=(t == nt - 1),
                    )
                for ac in range(ACH):
                    at_ps = l1ps.tile([P, P], BF16, tag="at_ps", bufs=3)
                    nc.tensor.transpose(at_ps[:], adj_bf[:, ts(ac)], ident[:])
                    nc.vector.tensor_copy(adjT4[:, ac, t, :], at_ps[:])

            # evacuate Q partials while qps pool is still open
            q_sb = persist.tile([P, ACH * FA], F32)
            q_sb3 = q_sb[:].rearrange("p (c f) -> p c f", c=ACH)
            for ac in range(ACH):
                nc.vector.tensor_copy(q_sb3[:, ac, :], q_ps[ac][:])

        # ---- all-reduce Q ----
        q_in = dram.tile([ACH * P, FA], F32)
        q_out = dram.tile([ACH * P, FA], F32)
        nc.sync.dma_start(
            out=q_in[:].rearrange("(c p) f -> p c f", p=P), in_=q_sb3)
        nc.gpsimd.collective_compute(
            "AllReduce",
            mybir.AluOpType.add,
            replica_groups=[list(range(n_cores))],
            ins=[q_in.opt()],
            outs=[q_out.opt()],
        )
        qr_sb = persist.tile([P, ACH * FA], F32)
        qr3 = qr_sb[:].rearrange("p (c f) -> p c f", c=ACH)
        nc.sync.dma_start(
            out=qr3, in_=q_out[:].rearrange("(c p) f -> p c f", p=P))

        # ---- msg = (Q @ W.T) / (colsum + eps), ones col appended ----
        rcol = persist.tile([P, ACH], F32)
        nc.vector.tensor_scalar_add(rcol[:], qr3[:, :, F], EPS)
        nc.vector.reciprocal(rcol[:], rcol[:])

        qr_bf = persist.tile([P, ACH * FA], BF16)
        nc.scalar.copy(qr_bf[:], qr_sb[:])
        qrb3 = qr_bf[:].rearrange("p (c f) -> p c f", c=ACH)

        qt_bf = persist.tile([P, 2 * APAD], BF16)   # [fi%128, (fi_half, a)]
        qt3 = qt_bf[:].rearrange("p (c a) -> p c a", c=2)
        with tc.tile_pool(name="t2ps", bufs=2, space="PSUM") as t2ps:
            for ac in range(ACH):
                for fh in range(2):
                    tp = t2ps.tile([P, P], BF16, tag="tp", bufs=2)
                    nc.tensor.transpose(
                        tp[:], qrb3[:, ac, ts(fh)], ident[:])
                    nc.vector.tensor_copy(qt3[:, fh, ts(ac)], tp[:])

        msg_bf = persist.tile([P, ACH * FA], BF16)  # [a%128, (a_chunk, f+1)]
        msg3 = msg_bf[:].rearrange("p (c f) -> p c f", c=ACH)
        with tc.tile_pool(name="mps", bufs=1, space="PSUM") as mpsp:
            for ac in range(ACH):
                mps = mpsp.tile([P, F], F32, tag=f"m{ac}", name=f"mps{ac}")
                for fh in range(2):
                    nc.tensor.matmul(
                        mps[:], qt3[:, fh, ts(ac)], wt_bf3[:, fh, :],
                        start=(fh == 0), stop=(fh == 1),
                    )
                nc.scalar.activation(
                    msg3[:, ac, 0:F], mps[:],
                    mybir.ActivationFunctionType.Copy,
                    scale=rcol[:, ac:ac + 1],
                )
            nc.vector.memset(msg3[:, :, F], 1.0)

        # ---- loop 3: out = (adj @ [msg|1]) / (rowsum + eps) ----
        with tc.tile_pool(name="l3ps", bufs=3, space="PSUM") as l3ps, \
             tc.tile_pool(name="l3", bufs=1) as l3:
            for t in range(nt):
                ops = l3ps.tile([P, FA], F32, tag="ops", bufs=3)
                for ac in range(ACH):
                    nc.tensor.matmul(
                        ops[:], adjT4[:, ac, t, :], msg3[:, ac, :],
                        start=(ac == 0), stop=(ac == ACH - 1),
                    )
                rr = l3.tile([P, 1], F32, tag="rr", bufs=3)
                nc.vector.tensor_scalar_add(rr[:], ops[:, F:FA], EPS)
                nc.vector.reciprocal(rr[:], rr[:])
                o_sb = l3.tile([P, F], F32, tag="o_sb", bufs=3)
                nc.scalar.activation(
                    o_sb[:], ops[:, 0:F],
                    mybir.ActivationFunctionType.Copy, scale=rr[:],
                )
                nc.sync.dma_start(out=out_d[ts(t), :], in_=o_sb[:])


# ---------------------------------------------------------------------------
# host side
# ---------------------------------------------------------------------------

_NC_CACHE = {}


def _get_nc(n_tiles=T_FULL, n_cores=CORES):
    key = (n_tiles, n_cores)
    if key not in _NC_CACHE:
        _NC_CACHE[key] = build(n_tiles, n_cores)
    return _NC_CACHE[key]


def shard_inputs(input, adj, W, n_tiles=T_FULL, n_cores=CORES):
    n = input.shape[0]
    rows = n_tiles * P
    total = rows * n_cores
    inp_aug = np.zeros((total, FA), dtype=np.float32)
    inp_aug[:n, :F] = input
    inp_aug[:n, F] = 1.0
    adj_p = np.zeros((total, APAD), dtype=np.float32)
    adj_p[:n, :A] = adj
    w = np.ascontiguousarray(W, dtype=np.float32)
    return [
        {
            "input": np.ascontiguousarray(inp_aug[c * rows:(c + 1) * rows]),
            "adj": np.ascontiguousarray(adj_p[c * rows:(c + 1) * rows]),
            "W": w,
        }
        for c in range(n_cores)
    ]


def kernel(input, adj, W):
    input = np.asarray(input, dtype=np.float32)
    adj = np.asarray(adj, dtype=np.float32)
    W = np.asarray(W, dtype=np.float32)
    nc = _get_nc()
    in_maps = shard_inputs(input, adj, W)
    res = run_bass_kernel_spmd(nc, in_maps, core_ids=list(range(CORES)))
    out = np.concatenate([res.results[c]["out"] for c in range(CORES)], axis=0)
    return np.ascontiguousarray(out[:input.shape[0]])


# revision 5
# speedup vs baseline: 1.1055x; 1.1055x over previous
"""AnchorGCN layer on 8 TRN2 NeuronCores.

reference:
    support = input @ W.T                         # [N, F]
    anchor_diff = adj / (colsum(adj) + eps)       # [N, A]
    node_diff   = adj / (rowsum(adj) + eps)       # [N, A]
    out = node_diff @ (anchor_diff.T @ support)   # [N, F]

Distributed formulation (rows of input/adj sharded across 8 cores):
    Q    = adj_shard.T @ [input_shard | 1]        # [A, F+1] per-core partial
           (col F of Q is the per-core colsum partial)
    QT   = AllReduce(Q^T)                         # only collective: 526 KB
    msg  = (Q[:, :F] @ W.T) / (colsum + eps)      # [A, F], computed per core
    out  = (adj_shard @ msg) / (rowsum + eps)     # rowsum free via accum_out

Matmuls run in bf16 (f32 PSUM accumulation); normalizations and the
all-reduce in f32. adj is transposed on-chip via TensorE (needed as the
stationary operand for the final matmul, which contracts over anchors);
the transposes are emitted after loop 1 so they fill the all-reduce
window with PE work.
"""

import numpy as np

import concourse.bacc as bacc
import concourse.mybir as mybir
import concourse.tile as tile
from concourse.bass_utils import run_bass_kernel_spmd
from concourse.masks import make_identity

F32 = mybir.dt.float32
BF16 = mybir.dt.bfloat16
COPY = mybir.ActivationFunctionType.Copy

N, A, F = 50000, 500, 256
EPS = 1e-12
CORES = 8
P = 128
APAD = 512            # anchors padded 500 -> 512 (4 chunks of 128)
FA = F + 1            # input gets a ones column appended
T_FULL = 49           # node tiles per core: 8*49*128 = 50176 >= 50000
ACH = APAD // P       # 4 anchor chunks
GRP = 4               # node tiles per DMA batch


def build(n_tiles: int = T_FULL, n_cores: int = CORES):
    nt = n_tiles
    rows = nt * P
    nc = bacc.Bacc("TRN2", target_bir_lowering=False, debug=False,
                   num_devices=n_cores)

    inp_d = nc.dram_tensor("input", [rows, FA], F32, kind="ExternalInput")
    adj_d = nc.dram_tensor("adj", [rows, APAD], F32, kind="ExternalInput")
    w_d = nc.dram_tensor("W", [F, F], F32, kind="ExternalInput")
    out_d = nc.dram_tensor("out", [rows, F], F32, kind="ExternalOutput")

    with tile.TileContext(nc) as tc:
        _build_tc(tc, nc, inp_d, adj_d, w_d, out_d, nt, n_cores)
    nc.compile()
    return nc


def _build_tc(tc, nc, inp_d, adj_d, w_d, out_d, nt, n_cores):
    ts = lambda i: slice(i * P, (i + 1) * P)
    inp_t = inp_d.ap().rearrange("(t p) f -> p t f", p=P)   # [128, nt, 257]
    adj_t = adj_d.ap().rearrange("(t p) a -> p t a", p=P)   # [128, nt, 512]
    out_t = out_d.ap().rearrange("(t p) f -> p t f", p=P)   # [128, nt, 256]

    with tc.tile_pool(name="const", bufs=1) as const, \
         tc.tile_pool(name="persist", bufs=1) as persist, \
         tc.tile_pool(name="dram", bufs=1, space="DRAM") as dram:

        ident = const.tile([P, P], BF16)
        make_identity(nc, ident[:])
        ident32 = const.tile([P, P], F32)
        make_identity(nc, ident32[:])

        # ---- W -> W^T (bf16), laid out [fi%128, (fi_half, fo)] ----
        w_sb = const.tile([P, 2 * F], F32)
        nc.sync.dma_start(
            out=w_sb[:].rearrange("p (c f) -> p c f", c=2),
            in_=w_d.ap().rearrange("(c p) f -> p c f", p=P),
        )
        w_bf = const.tile([P, 2 * F], BF16)
        nc.scalar.copy(w_bf[:], w_sb[:])
        wt_bf = const.tile([P, 2 * F], BF16)
        w_bf3 = w_bf[:].rearrange("p (c f) -> p c f", c=2)
        wt_bf3 = wt_bf[:].rearrange("p (c f) -> p c f", c=2)
        with tc.tile_pool(name="wt_ps", bufs=2, space="PSUM") as wtp:
            for foh in range(2):
                for fih in range(2):
                    w_ps = wtp.tile([P, P], BF16, tag="w_ps", bufs=2)
                    nc.tensor.transpose(
                        w_ps[:], w_bf3[:, foh, ts(fih)], ident[:])
                    nc.vector.tensor_copy(wt_bf3[:, fih, ts(foh)], w_ps[:])

        # persistent bf16 copies of the shard
        adj_bf = persist.tile([P, nt * APAD], BF16)      # [p, (t, a)]
        adj_bf3 = adj_bf[:].rearrange("p (t a) -> p t a", t=nt)
        adjT = persist.tile([P, ACH * nt * P], BF16)     # [a%128, (ac, t, n)]
        adjT4 = adjT[:].rearrange("p (c t n) -> p c t n", c=ACH, t=nt)
        rowsum = persist.tile([P, nt], F32)
        rrow = persist.tile([P, nt], F32)

        groups = [list(range(g, min(g + GRP, nt))) for g in range(0, nt, GRP)]

        # ================= loop 1: DMA + cast + Q accumulation ===========
        with tc.tile_pool(name="qps", bufs=1, space="PSUM") as qps, \
             tc.tile_pool(name="l1ps", bufs=2, space="PSUM") as l1ps, \
             tc.tile_pool(name="qtps", bufs=1, space="PSUM") as qtps, \
             tc.tile_pool(name="l1", bufs=1) as l1:
            q_ps = [qps.tile([P, FA], F32, tag=f"q{i}", name=f"q_ps{i}")
                    for i in range(ACH)]
            for grp in groups:
                g0, gl = grp[0], len(grp)
                in4 = l1.tile([P, gl * FA], F32, tag="in4", bufs=3)
                nc.sync.dma_start(
                    out=in4[:].rearrange("p (t f) -> p t f", t=gl),
                    in_=inp_t[:, g0:g0 + gl, :])
                adj4 = l1.tile([P, gl * APAD], F32, tag="adj4", bufs=2)
                nc.sync.dma_start(
                    out=adj4[:].rearrange("p (t a) -> p t a", t=gl),
                    in_=adj_t[:, g0:g0 + gl, :])
                in4v = in4[:].rearrange("p (t f) -> p t f", t=gl)
                adj4v = adj4[:].rearrange("p (t a) -> p t a", t=gl)
                for j, t in enumerate(grp):
                    # cast + free rowsum on ACT
                    nc.scalar.activation(
                        adj_bf3[:, t, :], adj4v[:, j, :], COPY,
                        accum_out=rowsum[:, t:t + 1])
                    in_bf = l1.tile([P, FA], BF16, tag="in_bf", bufs=8)
                    nc.vector.tensor_copy(in_bf[:], in4v[:, j, :])
                    for ac in range(ACH):
                        nc.tensor.matmul(
                            q_ps[ac][:], adj_bf3[:, t, ts(ac)], in_bf[:],
                            start=(t == 0), stop=(t == nt - 1),
                        )

            # 1/(rowsum+eps) for loop 3
            nc.vector.tensor_scalar_add(rrow[:], rowsum[:], EPS)
            nc.vector.reciprocal(rrow[:], rrow[:])

            # ---- evacuate Q, transpose it (f32, exact), ship to AR ----
            q_sb = persist.tile([P, ACH * F], F32)
            q_sb3 = q_sb[:].rearrange("p (c f) -> p c f", c=ACH)
            cs_sb = persist.tile([P, ACH], F32)
            for ac in range(ACH):
                nc.vector.tensor_copy(q_sb3[:, ac, :], q_ps[ac][:, 0:F])
                nc.vector.tensor_copy(
                    cs_sb[:, ac:ac + 1], q_ps[ac][:, F:FA])

            # single contiguous AR payload: [p, (fh, a)] Q^T + 4 colsum cols
            ar_sb = persist.tile([P, 2 * APAD + ACH], F32)
            qt3 = ar_sb[:, 0:2 * APAD].rearrange("p (c a) -> p c a", c=2)
            for fh in range(2):
                qt_ps = qtps.tile([P, APAD], F32, tag="qt", bufs=2)
                for ac in range(ACH):
                    nc.tensor.transpose(
                        qt_ps[:, ts(ac)], q_sb3[:, ac, ts(fh)], ident32[:])
                nc.vector.tensor_copy(qt3[:, fh, :], qt_ps[:])
            nc.vector.tensor_copy(ar_sb[:, 2 * APAD:], cs_sb[:])

            q_in = dram.tile([P, 2 * APAD + ACH], F32)
            q_out = dram.tile([P, 2 * APAD + ACH], F32)
            nc.sync.dma_start(out=q_in[:, :], in_=ar_sb[:])
            nc.gpsimd.collective_compute(
                "AllReduce",
                mybir.AluOpType.add,
                replica_groups=[list(range(n_cores))],
                ins=[q_in.opt()],
                outs=[q_out.opt()],
            )

            # ---- adj transposes: PE/DVE work that fills the AR window ----
            for t in range(nt):
                at_ps = l1ps.tile([P, APAD], BF16, tag="at_ps", bufs=2)
                for ac in range(ACH):
                    nc.tensor.transpose(
                        at_ps[:, ts(ac)], adj_bf3[:, t, ts(ac)], ident[:])
                nc.vector.tensor_copy(adjT4[:, :, t, :], at_ps[:])

        # ---- AR result -> msg = (Q @ W.T) / (colsum + eps) ----
        ar2_sb = persist.tile([P, 2 * APAD + ACH], F32)
        nc.sync.dma_start(out=ar2_sb[:], in_=q_out[:, :])
        rcol = persist.tile([P, ACH], F32)
        nc.vector.tensor_scalar_add(rcol[:], ar2_sb[:, 2 * APAD:], EPS)
        nc.vector.reciprocal(rcol[:], rcol[:])

        qt_bf = persist.tile([P, 2 * APAD], BF16)
        nc.scalar.copy(qt_bf[:], ar2_sb[:, 0:2 * APAD])
        qtb3 = qt_bf[:].rearrange("p (c a) -> p c a", c=2)

        msg_bf = persist.tile([P, ACH * F], BF16)   # [a%128, (ac, f)]
        msg3 = msg_bf[:].rearrange("p (c f) -> p c f", c=ACH)
        with tc.tile_pool(name="mps", bufs=1, space="PSUM") as mpsp:
            for ac in range(ACH):
                mps = mpsp.tile([P, F], F32, tag=f"m{ac}", name=f"mps{ac}")
                for fh in range(2):
                    nc.tensor.matmul(
                        mps[:], qtb3[:, fh, ts(ac)], wt_bf3[:, fh, :],
                        start=(fh == 0), stop=(fh == 1),
                    )
                nc.scalar.activation(
                    msg3[:, ac, :], mps[:], COPY, scale=rcol[:, ac:ac + 1])

        # ====== loop 3: out = (adj @ msg) / (rowsum + eps) ======
        with tc.tile_pool(name="l3ps", bufs=6, space="PSUM") as l3ps, \
             tc.tile_pool(name="l3", bufs=1) as l3:
            for grp in groups:
                g0, gl = grp[0], len(grp)
                o4 = l3.tile([P, gl * F], F32, tag="o4", bufs=3)
                o4v = o4[:].rearrange("p (t f) -> p t f", t=gl)
                for j, t in enumerate(grp):
                    ops = l3ps.tile([P, F], F32, tag="ops", bufs=6)
                    for ac in range(ACH):
                        nc.tensor.matmul(
                            ops[:], adjT4[:, ac, t, :], msg3[:, ac, :],
                            start=(ac == 0), stop=(ac == ACH - 1),
                        )
                    nc.scalar.activation(
                        o4v[:, j, :], ops[:], COPY, scale=rrow[:, t:t + 1])
                nc.sync.dma_start(
                    out=out_t[:, g0:g0 + gl, :], in_=o4v)


# ---------------------------------------------------------------------------
# host side
# ---------------------------------------------------------------------------

_NC_CACHE = {}


def _get_nc(n_tiles=T_FULL, n_cores=CORES):
    key = (n_tiles, n_cores)
    if key not in _NC_CACHE:
        _NC_CACHE[key] = build(n_tiles, n_cores)
    return _NC_CACHE[key]


def shard_inputs(input, adj, W, n_tiles=T_FULL, n_cores=CORES):
    n = input.shape[0]
    rows = n_tiles * P
    total = rows * n_cores
    inp_aug = np.zeros((total, FA), dtype=np.float32)
    inp_aug[:n, :F] = input
    inp_aug[:n, F] = 1.0
    adj_p = np.zeros((total, APAD), dtype=np.float32)
    adj_p[:n, :A] = adj
    w = np.ascontiguousarray(W, dtype=np.float32)
    return [
        {
            "input": np.ascontiguousarray(inp_aug[c * rows:(c + 1) * rows]),
            "adj": np.ascontiguousarray(adj_p[c * rows:(c + 1) * rows]),
            "W": w,
        }
        for c in range(n_cores)
    ]


def kernel(input, adj, W):
    input = np.asarray(input, dtype=np.float32)
    adj = np.asarray(adj, dtype=np.float32)
    W = np.asarray(W, dtype=np.float32)
    nc = _get_nc()
    in_maps = shard_inputs(input, adj, W)
    res = run_bass_kernel_spmd(nc, in_maps, core_ids=list(range(CORES)))
    out = np.concatenate([res.results[c]["out"] for c in range(CORES)], axis=0)
    return np.ascontiguousarray(out[:input.shape[0]])
